# revision 37
# baseline (speedup 1.0000x reference)
"""Trainium2 Bass kernel for windowed multi-head attention.

Model (from the reference):
  x: [bt=16, n=1024, c=512], t=8 -> b=2 temporal batches of t frames.
  q/k/v = x @ W{q,k,v}.T + b; tokens regrouped into 4 spatial 2x2 windows
  of a 32x32 grid; attention runs independently per (batch, window, head)
  over S = t*16*16 = 2048 tokens with head dim 64; output projection Wo.

Sharding: one (batch, window) pair per NeuronCore -> 8 independent shards,
no collectives. The host extracts each shard's 2048 tokens (already in
window order), pre-transposes x to [c, S] so every on-device matmul has
its contraction dim on partitions, and scatters the per-core results back.

Per-core dataflow (S=2048, c=512, 8 heads x ch=64):
  qT/kT = W @ xT      (fp32 matmuls, cast to bf16 in SBUF; 1/8 folded in Wq)
  v     = xT.T @ WvT  (natural [token, c] layout, bf16, +ones column/head)
  per head pair, per 512-wide q chunk, per 128-key tile:
    scoresT[key, q] = kT_h^T.T @ qT_h   (two heads row-packed in the PE)
    pT = exp(scoresT)                   (ScalarE, PSUM->SBUF, bf16)
    att_unnorm^T | rowsum = v_aug.T.T @ pT accumulated over key tiles
  softmax denominators ride along as a ones-column of v (row 64 of the
  PV accumulator); normalization = DVE reciprocal on lane 64 + GpSimd
  partition-broadcast + one fused multiply during the PSUM->SBUF copy.
  Finally out = attT.T @ WoT per 64-row head chunk.
"""

import os
import sys
import numpy as np

for _p in ("/opt/trn_rl_repo", "/root/.axon_site", "/root/.axon_site/_ro/trn_rl_repo",
           "/root/.axon_site/_ro/pypackages"):
    if os.path.isdir(_p) and _p not in sys.path:
        sys.path.append(_p)

import ml_dtypes

import concourse.bass as bass
import concourse.bacc as bacc
import concourse.mybir as mybir
import concourse.tile as tile
from concourse.bass_utils import run_bass_kernel_spmd

N_CORES = 8
HEAD = 8
CH = 64
C = 512
S = 2048
P = 128
KT = C // P          # 4 contraction tiles of 128 over c
TOKT = S // P        # 16 token tiles of 128
QC = S // 512        # 4 query chunks of 512
NKT = S // P         # 16 key tiles of 128

F32 = mybir.dt.float32
BF16 = mybir.dt.bfloat16
F8 = mybir.dt.float8e4
V_SCALE = 8.0      # v and the softmax ones-column share this scale; the
                   # att = num/den ratio cancels it (keeps v out of fp8
                   # subnormals)

LAST_EXEC_NS = None
LAST_RESULTS = None
_COMPILED = None
DEBUG_TAPS = False


def _build_nc():
    nc = bacc.Bacc("TRN2", target_bir_lowering=False, debug=False,
                   num_devices=N_CORES)

    xT_d = nc.dram_tensor("xT", [C, S], BF16, kind="ExternalInput").ap()
    wq_d = nc.dram_tensor("wqT", [C, C], BF16, kind="ExternalInput").ap()
    wk_d = nc.dram_tensor("wkT", [C, C], BF16, kind="ExternalInput").ap()
    wv_d = nc.dram_tensor("wvT", [C, C], BF16, kind="ExternalInput").ap()
    wo_d = nc.dram_tensor("woT", [C, C], BF16, kind="ExternalInput").ap()
    out_d = nc.dram_tensor("out", [S, C], F32, kind="ExternalOutput").ap()

    dbg = None
    if DEBUG_TAPS:
        dbg = {
            "d_qT0": nc.dram_tensor("d_qT0", [P, S], BF16, kind="ExternalOutput").ap(),
            "d_kT0": nc.dram_tensor("d_kT0", [P, S], BF16, kind="ExternalOutput").ap(),
            "d_v0": nc.dram_tensor("d_v0", [P, HEAD, CH + 1], BF16, kind="ExternalOutput").ap(),
            "d_pT": nc.dram_tensor("d_pT", [P, 1024], BF16, kind="ExternalOutput").ap(),
            "d_aps": nc.dram_tensor("d_aps", [P, 512], F32, kind="ExternalOutput").ap(),
            "d_rec": nc.dram_tensor("d_rec", [1, 512], F32, kind="ExternalOutput").ap(),
            "d_bc": nc.dram_tensor("d_bc", [CH, 512], F32, kind="ExternalOutput").ap(),
            "d_attT0": nc.dram_tensor("d_attT0", [CH, S], BF16, kind="ExternalOutput").ap(),
        }

    with tile.TileContext(nc) as tc:
        _emit(tc, nc, xT_d, wq_d, wk_d, wv_d, wo_d, out_d, dbg)
    nc.compile()
    return nc


def _emit(tc, nc, xT_d, wq_d, wk_d, wv_d, wo_d, out_d, dbg=None):
    from contextlib import ExitStack
    ctx = ExitStack()
    persist = ctx.enter_context(tc.tile_pool(name="persist", bufs=1))
    xw_pool = ctx.enter_context(tc.tile_pool(name="xw", bufs=1))
    spsum = ctx.enter_context(tc.tile_pool(name="spsum", bufs=2, space="PSUM"))
    # one rotating set of [128, 512] PSUM banks shared by the PV
    # accumulators, projection psums and output psums
    apsum = ctx.enter_context(tc.tile_pool(name="apsum", bufs=4, space="PSUM"))
    ppool = ctx.enter_context(tc.tile_pool(name="ppool", bufs=2 * NKT + 2))
    npool = ctx.enter_context(tc.tile_pool(name="npool", bufs=3))
    owork = ctx.enter_context(tc.tile_pool(name="owork", bufs=3))

    qT_sb = [persist.tile([P, S], BF16, tag=f"qT{i}", name=f"qT{i}") for i in range(KT)]
    kT_sb = [persist.tile([P, S], BF16, tag=f"kT{i}", name=f"kT{i}") for i in range(KT)]
    # v with an interleaved ones column per head: [tok, head, 65]
    v_sb = [persist.tile([P, HEAD, CH + 1], BF16, tag=f"v{i}", name=f"v{i}")
            for i in range(TOKT)]
    wo_sb = persist.tile([P, KT, C], BF16, tag="w_wo", name="w_wo")
    # attention outputs, head PAIRS packed on partitions: attT_m [128, S]
    attT_sb = [persist.tile([P, S], BF16, tag=f"attT{i}", name=f"attT{i}")
               for i in range(HEAD // 2)]

    # ---- input DMAs, chunked in first-use order ----------------------
    # first k0/q0 projection group needs wk/wq chunk 0 + the first 512
    # token-columns of every xT tile; land exactly those first.
    w_sb = {nm: xw_pool.tile([P, KT, C], BF16, tag=f"w_{nm}", name=f"w_{nm}")
            for nm in ("wq", "wk", "wv")}
    xT_sb = [xw_pool.tile([P, S], BF16, tag=f"xT{i}", name=f"xT{i}")
             for i in range(KT)]
    wv_q = {"wq": wq_d, "wk": wk_d, "wv": wv_d}

    def dma_w_chunk(nm, ko):
        nc.sync.dma_start(w_sb[nm][:, ko, :],
                          wv_q[nm][ko * P:(ko + 1) * P, :])

    def dma_x_chunk(i, cchunk):
        cs = slice(cchunk * 512, (cchunk + 1) * 512)
        nc.sync.dma_start(xT_sb[i][:, cs], xT_d[i * P:(i + 1) * P, cs])

    for ko in range(KT):
        dma_w_chunk("wk", ko)
        dma_w_chunk("wq", ko)
    for i in range(KT):
        dma_x_chunk(i, 0)
    for cchunk in (1, 2, 3):
        for i in range(KT):
            dma_x_chunk(i, cchunk)
    for ko in range(KT):
        dma_w_chunk("wv", ko)
    nc.sync.dma_start(wo_sb[:], wo_d.rearrange("(ko p) j -> p ko j", p=P))

    # ---- building blocks (emitted on demand, PSUM from the shared
    #      "ap" rotation so they interleave with attention) ------------
    def emit_qk_proj(nm, jt, tcs=tuple(range(QC))):
        dst = qT_sb if nm == "wq" else kT_sb
        w = w_sb[nm]
        for tc_i in tcs:
            ps = apsum.tile([P, 512], F32, tag="ap", name="proj_ps")
            for kt in range(KT):
                nc.tensor.matmul(
                    ps[:],
                    lhsT=w[:, kt, jt * P:(jt + 1) * P],
                    rhs=xT_sb[kt][:, tc_i * 512:(tc_i + 1) * 512],
                    start=(kt == 0), stop=(kt == KT - 1))
            nc.vector.tensor_copy(
                dst[jt][:, tc_i * 512:(tc_i + 1) * 512], ps[:])

    def emit_v_proj(tt):
        w = w_sb["wv"]
        ps = apsum.tile([P, 512], F32, tag="ap", name="vproj_ps")
        for kt in range(KT):
            nc.tensor.matmul(
                ps[:],
                lhsT=xT_sb[kt][:, tt * P:(tt + 1) * P],
                rhs=w[:, kt, :],
                start=(kt == 0), stop=(kt == KT - 1))
        nc.vector.tensor_copy(
            v_sb[tt][:, :, 0:CH],
            ps.rearrange("p (h c) -> p h c", h=HEAD))
        nc.vector.memset(v_sb[tt][:, :, CH], 1.0)

    def emit_out_proj(tt):
        ps = apsum.tile([P, 512], F32, tag="ap", name="out_ps")
        for kc in range(KT):
            nc.tensor.matmul(
                ps[:],
                lhsT=attT_sb[kc][:, tt * P:(tt + 1) * P],
                rhs=wo_sb[:, kc, :],
                start=(kc == 0), stop=(kc == KT - 1))
        ot = owork.tile([P, C], F32, tag="out", name="out_sb")
        nc.vector.tensor_copy(ot[:], ps[:])
        nc.sync.dma_start(out_d[tt * P:(tt + 1) * P, :], ot[:])

    # Small PE work groups (~4 matmuls each) are queued and drained one
    # per key-tile inside the scores loop: the PV of iteration i, the
    # remaining q/k projections and the output projection all interleave
    # with iteration i+1's scores so ScalarE's exp stream never starves.
    pending = []

    def drain_one():
        if pending:
            pending.pop(0)()

    def scores_block(m, qc):
        kT_m, qT_m = kT_sb[m], qT_sb[m]
        qs = slice(qc * 512, (qc + 1) * 512)
        pts = []
        for kt in range(NKT):
            ks = slice(kt * P, (kt + 1) * P)
            sp = spsum.tile([P, 1024], F32, tag="sp", name="sp")
            # two heads row-packed: contraction rows 0-63 / 64-127
            nc.tensor.matmul(sp[:, 0:512],
                             lhsT=kT_m[0:CH, ks], rhs=qT_m[0:CH, qs])
            nc.tensor.matmul(sp[:, 512:1024],
                             lhsT=kT_m[CH:P, ks], rhs=qT_m[CH:P, qs])
            pt = ppool.tile([P, 1024], BF16, tag="pT", name="pT")
            nc.scalar.activation(pt[:], sp[:],
                                 mybir.ActivationFunctionType.Exp)
            pts.append(pt)
            drain_one()
        return pts

    def make_pv_groups(m, qc, pts):
        """8 pending groups of 4 accumulating PV matmuls; PSUM tiles are
        allocated lazily at the first group's emission."""
        aps = []

        def group(g):
            if not aps:
                aps.extend(apsum.tile([P, 512], F32, tag="ap", name="ap")
                           for _ in range(2))
            for kt in (2 * g, 2 * g + 1):
                for j in range(2):
                    h = 2 * m + j
                    nc.tensor.matmul(
                        aps[j][0:CH + 1, :],
                        lhsT=v_sb[kt][:, h, :],
                        rhs=pts[kt][:, j * 512:(j + 1) * 512],
                        start=(kt == 0), stop=(kt == NKT - 1))

        return [lambda g=g: group(g) for g in range(NKT // 2)], aps

    def norm_block(m, qc, aps):
        qs = slice(qc * 512, (qc + 1) * 512)
        for j in range(2):
            # partition_broadcast + custom DVE ops need base-0 APs on
            # HW: shift-copy the rowsum row 64 -> 0 first.
            rs = npool.tile([1, 512], F32, tag="rs", name="rs")
            nc.vector.tensor_copy(rs[0:1, :], aps[j][CH:CH + 1, :])
            rec = npool.tile([1, 512], F32, tag="rec", name="rec")
            nc.vector.reciprocal_approx_fast(rec[0:1, :], rs[0:1, :])
            bc = npool.tile([CH, 512], F32, tag="bc", name="bc")
            nc.gpsimd.partition_broadcast(bc[:], rec[0:1, :])
            # odd head writes lanes 64-127 (DVE partition shift)
            nc.vector.tensor_mul(attT_sb[m][j * CH:(j + 1) * CH, qs],
                                 aps[j][0:CH, :], bc[:])
            if dbg is not None and m == 0 and qc == 0 and j == 0:
                aps_sb = npool.tile([P, 512], F32, tag="dbg_aps",
                                    name="aps_sb")
                nc.vector.tensor_copy(aps_sb[:], aps[0][:])
                nc.sync.dma_start(dbg["d_aps"][:], aps_sb[:])
                nc.sync.dma_start(dbg["d_rec"][:], rec[0:1, :])
                nc.sync.dma_start(dbg["d_bc"][:], bc[:])

    # pending additions keyed by the iteration whose scores drain them
    adds = {
        (0, 2): [lambda t=i: emit_qk_proj("wq", 1, tcs=(t,)) for i in range(QC)],
        (0, 3): [lambda t=i: emit_qk_proj("wk", 1, tcs=(t,)) for i in range(QC)],
        (1, 0): [lambda t=i: emit_qk_proj("wq", 2, tcs=(t,)) for i in range(QC)]
              + [lambda t=i: emit_qk_proj("wk", 2, tcs=(t,)) for i in range(QC)],
        (1, 2): [lambda t=i: emit_qk_proj("wq", 3, tcs=(t,)) for i in range(QC)]
              + [lambda t=i: emit_qk_proj("wk", 3, tcs=(t,)) for i in range(QC)],
        (3, 1): [lambda t=tt: emit_out_proj(t) for tt in range(0, 4)],
        (3, 2): [lambda t=tt: emit_out_proj(t) for tt in range(4, 8)],
        (3, 3): [lambda t=tt: emit_out_proj(t) for tt in range(8, 12)],
    }

    # ---- prologue: minimal projection, then v under the first exps ---
    emit_qk_proj("wk", 0, tcs=(0,))
    emit_qk_proj("wq", 0, tcs=(0,))
    pending.extend([lambda t=i: emit_qk_proj("wk", 0, tcs=(t,))
                    for i in (1, 2, 3)])
    pending.append(lambda: emit_qk_proj("wq", 0, tcs=(1,)))
    pts_prev = scores_block(0, 0)
    if dbg is not None:
        nc.sync.dma_start(dbg["d_pT"][:], pts_prev[0][:])
    pending.extend([lambda t=i: emit_qk_proj("wq", 0, tcs=(t,))
                    for i in (2, 3)])
    for tt in range(TOKT):
        emit_v_proj(tt)

    # ---- software-pipelined main loop: scores(i+1) drains PV(i) + its
    # normalization (queued right behind the PV groups) -----------------
    iters = [(m, qc) for m in range(HEAD // 2) for qc in range(QC)]
    groups, aps = make_pv_groups(0, 0, pts_prev)
    pending.extend(groups)
    pending.append(lambda m=0, qc=0, a=aps: norm_block(m, qc, a))
    for (m, qc) in iters[1:]:
        pending.extend(adds.get((m, qc), []))
        pts = scores_block(m, qc)
        groups, aps = make_pv_groups(m, qc, pts)
        pending.extend(groups)
        pending.append(lambda m=m, qc=qc, a=aps: norm_block(m, qc, a))

    while pending:
        drain_one()

    for tt in range(12, 16):
        emit_out_proj(tt)

    if dbg is not None:
        nc.sync.dma_start(dbg["d_qT0"][:], qT_sb[0][:])
        nc.sync.dma_start(dbg["d_kT0"][:], kT_sb[0][:])
        nc.sync.dma_start(dbg["d_v0"][:], v_sb[0][:])
        nc.sync.dma_start(dbg["d_attT0"][:], attT_sb[0][0:CH, :])

    ctx.close()


def _numpy_fallback(x, Wq, bq, Wk, bk, Wv, bv, Wo, bo, t):
    """Exact replica of the reference in numpy (general path; only used for
    shapes/biases the tuned kernel doesn't handle)."""
    bt, n, c = x.shape
    b = bt // t
    head, ch = HEAD, c // HEAD
    hh = int(np.sqrt(n)); h2 = w2 = hh // 2
    q = x @ Wq.T + bq
    k = x @ Wk.T + bk
    v = x @ Wv.T + bv

    def win(z):
        z = z.reshape(b, t, 2, h2, 2, w2, head, ch)
        z = z.transpose(0, 2, 4, 6, 1, 3, 5, 7)
        return z.reshape(b, 4, head, t * h2 * w2, ch)

    q, k, v = win(q), win(k), win(v)
    s = (q @ k.transpose(0, 1, 2, 4, 3)) / np.sqrt(ch)
    s = s - s.max(-1, keepdims=True)
    p = np.exp(s)
    p = p / p.sum(-1, keepdims=True)
    att = p @ v
    att = att.reshape(b, 2, 2, head, t, h2, w2, ch)
    att = att.transpose(0, 4, 1, 5, 2, 6, 3, 7).reshape(bt, n, c)
    return (att @ Wo.T + bo).astype(x.dtype)


def kernel(x, Wq, bq, Wk, bk, Wv, bv, Wo, bo, t):
    global LAST_EXEC_NS, LAST_RESULTS, _COMPILED
    x = np.asarray(x); t = int(t)
    Wq, Wk, Wv, Wo = (np.asarray(a, np.float32) for a in (Wq, Wk, Wv, Wo))
    bq, bk, bv, bo = (np.asarray(a, np.float32) for a in (bq, bk, bv, bo))

    general = (
        x.shape != (16, 1024, 512) or t != 8
        or any(np.any(b) for b in (bq, bk, bv, bo))
    )
    if general:
        return _numpy_fallback(x, Wq, bq, Wk, bk, Wv, bv, Wo, bo, t)

    if _COMPILED is None:
        _COMPILED = _build_nc()
    nc = _COMPILED

    b = 16 // t
    # [b, 2, 2, t, 16, 16, c] spatial-window view of the token grid
    x6 = np.ascontiguousarray(
        x.reshape(b, t, 2, 16, 2, 16, C).transpose(0, 2, 4, 1, 3, 5, 6))
    wqT = (np.ascontiguousarray(Wq.T) * np.float32(1.0 / np.sqrt(CH))).astype(ml_dtypes.bfloat16)
    wkT = np.ascontiguousarray(Wk.T).astype(ml_dtypes.bfloat16)
    wvT = np.ascontiguousarray(Wv.T).astype(ml_dtypes.bfloat16)
    woT = np.ascontiguousarray(Wo.T).astype(ml_dtypes.bfloat16)

    in_maps = []
    for core in range(N_CORES):
        bb, wi, wj = core // 4, (core // 2) % 2, core % 2
        xw = x6[bb, wi, wj].reshape(S, C)
        in_maps.append({
            "xT": np.ascontiguousarray(xw.T).astype(ml_dtypes.bfloat16),
            "wqT": wqT, "wkT": wkT, "wvT": wvT, "woT": woT,
        })

    res = run_bass_kernel_spmd(nc, in_maps, list(range(N_CORES)))
    LAST_EXEC_NS = res.exec_time_ns
    LAST_RESULTS = res.results

    y6 = np.empty((b, 2, 2, t, 16, 16, C), np.float32)
    for core in range(N_CORES):
        bb, wi, wj = core // 4, (core // 2) % 2, core % 2
        y6[bb, wi, wj] = res.results[core]["out"].reshape(t, 16, 16, C)
    # invert the window view back to [bt, n, c]
    y = y6.transpose(0, 3, 1, 4, 2, 5, 6).reshape(16, 1024, C)
    return np.ascontiguousarray(y)


# revision 38
# speedup vs baseline: 1.0102x; 1.0102x over previous
"""Trainium2 Bass kernel for windowed multi-head attention.

Model (from the reference):
  x: [bt=16, n=1024, c=512], t=8 -> b=2 temporal batches of t frames.
  q/k/v = x @ W{q,k,v}.T + b; tokens regrouped into 4 spatial 2x2 windows
  of a 32x32 grid; attention runs independently per (batch, window, head)
  over S = t*16*16 = 2048 tokens with head dim 64; output projection Wo.

Sharding: one (batch, window) pair per NeuronCore -> 8 independent shards,
no collectives. The host extracts each shard's 2048 tokens (already in
window order), pre-transposes x to [c, S] so every on-device matmul has
its contraction dim on partitions, and scatters the per-core results back.

Per-core dataflow (S=2048, c=512, 8 heads x ch=64):
  qT/kT = W @ xT      (fp32 matmuls, cast to bf16 in SBUF; 1/8 folded in Wq)
  v     = xT.T @ WvT  (natural [token, c] layout, bf16, +ones column/head)
  per head pair, per 512-wide q chunk, per 128-key tile:
    scoresT[key, q] = kT_h^T.T @ qT_h   (two heads row-packed in the PE)
    pT = exp(scoresT)                   (ScalarE, PSUM->SBUF, bf16)
    att_unnorm^T | rowsum = v_aug.T.T @ pT accumulated over key tiles
  softmax denominators ride along as a ones-column of v (row 64 of the
  PV accumulator); normalization = DVE reciprocal on lane 64 + GpSimd
  partition-broadcast + one fused multiply during the PSUM->SBUF copy.
  Finally out = attT.T @ WoT per 64-row head chunk.
"""

import os
import sys
import numpy as np

for _p in ("/opt/trn_rl_repo", "/root/.axon_site", "/root/.axon_site/_ro/trn_rl_repo",
           "/root/.axon_site/_ro/pypackages"):
    if os.path.isdir(_p) and _p not in sys.path:
        sys.path.append(_p)

import ml_dtypes

import concourse.bass as bass
import concourse.bacc as bacc
import concourse.mybir as mybir
import concourse.tile as tile
from concourse.bass_utils import run_bass_kernel_spmd

N_CORES = 8
HEAD = 8
CH = 64
C = 512
S = 2048
P = 128
KT = C // P          # 4 contraction tiles of 128 over c
TOKT = S // P        # 16 token tiles of 128
QC = S // 512        # 4 query chunks of 512
NKT = S // P         # 16 key tiles of 128

F32 = mybir.dt.float32
BF16 = mybir.dt.bfloat16
F8 = mybir.dt.float8e4
V_SCALE = 8.0      # v and the softmax ones-column share this scale; the
                   # att = num/den ratio cancels it (keeps v out of fp8
                   # subnormals)

LAST_EXEC_NS = None
LAST_RESULTS = None
_COMPILED = None
DEBUG_TAPS = False


def _build_nc():
    nc = bacc.Bacc("TRN2", target_bir_lowering=False, debug=False,
                   num_devices=N_CORES)

    xT_d = nc.dram_tensor("xT", [C, S], BF16, kind="ExternalInput").ap()
    wq_d = nc.dram_tensor("wqT", [C, C], BF16, kind="ExternalInput").ap()
    wk_d = nc.dram_tensor("wkT", [C, C], BF16, kind="ExternalInput").ap()
    wv_d = nc.dram_tensor("wvT", [C, C], BF16, kind="ExternalInput").ap()
    wo_d = nc.dram_tensor("woT", [C, C], BF16, kind="ExternalInput").ap()
    out_d = nc.dram_tensor("out", [S, C], F32, kind="ExternalOutput").ap()

    dbg = None
    if DEBUG_TAPS:
        dbg = {
            "d_qT0": nc.dram_tensor("d_qT0", [P, S], BF16, kind="ExternalOutput").ap(),
            "d_kT0": nc.dram_tensor("d_kT0", [P, S], BF16, kind="ExternalOutput").ap(),
            "d_v0": nc.dram_tensor("d_v0", [P, HEAD, CH + 1], BF16, kind="ExternalOutput").ap(),
            "d_pT": nc.dram_tensor("d_pT", [P, 1024], BF16, kind="ExternalOutput").ap(),
            "d_aps": nc.dram_tensor("d_aps", [P, 512], F32, kind="ExternalOutput").ap(),
            "d_rec": nc.dram_tensor("d_rec", [1, 512], F32, kind="ExternalOutput").ap(),
            "d_bc": nc.dram_tensor("d_bc", [CH, 512], F32, kind="ExternalOutput").ap(),
            "d_attT0": nc.dram_tensor("d_attT0", [CH, S], BF16, kind="ExternalOutput").ap(),
        }

    with tile.TileContext(nc) as tc:
        _emit(tc, nc, xT_d, wq_d, wk_d, wv_d, wo_d, out_d, dbg)
    nc.compile()
    return nc


def _emit(tc, nc, xT_d, wq_d, wk_d, wv_d, wo_d, out_d, dbg=None):
    from contextlib import ExitStack
    ctx = ExitStack()
    persist = ctx.enter_context(tc.tile_pool(name="persist", bufs=1))
    xw_pool = ctx.enter_context(tc.tile_pool(name="xw", bufs=1))
    spsum = ctx.enter_context(tc.tile_pool(name="spsum", bufs=2, space="PSUM"))
    # one rotating set of [128, 512] PSUM banks shared by the PV
    # accumulators, projection psums and output psums
    apsum = ctx.enter_context(tc.tile_pool(name="apsum", bufs=4, space="PSUM"))
    ppool = ctx.enter_context(tc.tile_pool(name="ppool", bufs=2 * NKT + 2))
    npool = ctx.enter_context(tc.tile_pool(name="npool", bufs=3))
    owork = ctx.enter_context(tc.tile_pool(name="owork", bufs=3))

    qT_sb = [persist.tile([P, S], BF16, tag=f"qT{i}", name=f"qT{i}") for i in range(KT)]
    kT_sb = [persist.tile([P, S], BF16, tag=f"kT{i}", name=f"kT{i}") for i in range(KT)]
    # v with an interleaved ones column per head: [tok, head, 65]
    v_sb = [persist.tile([P, HEAD, CH + 1], BF16, tag=f"v{i}", name=f"v{i}")
            for i in range(TOKT)]
    wo_sb = persist.tile([P, KT, C], BF16, tag="w_wo", name="w_wo")
    # attention outputs, head PAIRS packed on partitions: attT_m [128, S]
    attT_sb = [persist.tile([P, S], BF16, tag=f"attT{i}", name=f"attT{i}")
               for i in range(HEAD // 2)]

    # ---- input DMAs in first-use order ------------------------------
    w_sb = {}
    for nm, d in (("wk", wk_d), ("wq", wq_d)):
        t = xw_pool.tile([P, KT, C], BF16, tag=f"w_{nm}", name=f"w_{nm}")
        nc.sync.dma_start(t[:], d.rearrange("(ko p) j -> p ko j", p=P))
        w_sb[nm] = t
    xT_sb = []
    for i in range(KT):
        t = xw_pool.tile([P, S], BF16, tag=f"xT{i}", name=f"xT{i}")
        nc.sync.dma_start(t[:], xT_d[i * P:(i + 1) * P, :])
        xT_sb.append(t)
    for nm, d in (("wv", wv_d),):
        t = xw_pool.tile([P, KT, C], BF16, tag=f"w_{nm}", name=f"w_{nm}")
        nc.sync.dma_start(t[:], d.rearrange("(ko p) j -> p ko j", p=P))
        w_sb[nm] = t
    nc.sync.dma_start(wo_sb[:], wo_d.rearrange("(ko p) j -> p ko j", p=P))

    # ---- building blocks (emitted on demand, PSUM from the shared
    #      "ap" rotation so they interleave with attention) ------------
    def emit_qk_proj(nm, jt, tcs=tuple(range(QC))):
        dst = qT_sb if nm == "wq" else kT_sb
        w = w_sb[nm]
        for tc_i in tcs:
            ps = apsum.tile([P, 512], F32, tag="ap", name="proj_ps")
            for kt in range(KT):
                nc.tensor.matmul(
                    ps[:],
                    lhsT=w[:, kt, jt * P:(jt + 1) * P],
                    rhs=xT_sb[kt][:, tc_i * 512:(tc_i + 1) * 512],
                    start=(kt == 0), stop=(kt == KT - 1))
            nc.vector.tensor_copy(
                dst[jt][:, tc_i * 512:(tc_i + 1) * 512], ps[:])

    def emit_v_proj(tt):
        w = w_sb["wv"]
        ps = apsum.tile([P, 512], F32, tag="ap", name="vproj_ps")
        for kt in range(KT):
            nc.tensor.matmul(
                ps[:],
                lhsT=xT_sb[kt][:, tt * P:(tt + 1) * P],
                rhs=w[:, kt, :],
                start=(kt == 0), stop=(kt == KT - 1))
        nc.vector.tensor_copy(
            v_sb[tt][:, :, 0:CH],
            ps.rearrange("p (h c) -> p h c", h=HEAD))
        nc.vector.memset(v_sb[tt][:, :, CH], 1.0)

    def emit_out_proj(tt):
        ps = apsum.tile([P, 512], F32, tag="ap", name="out_ps")
        for kc in range(KT):
            nc.tensor.matmul(
                ps[:],
                lhsT=attT_sb[kc][:, tt * P:(tt + 1) * P],
                rhs=wo_sb[:, kc, :],
                start=(kc == 0), stop=(kc == KT - 1))
        ot = owork.tile([P, C], F32, tag="out", name="out_sb")
        nc.vector.tensor_copy(ot[:], ps[:])
        nc.sync.dma_start(out_d[tt * P:(tt + 1) * P, :], ot[:])

    # Small PE work groups (~4 matmuls each) are queued and drained one
    # per key-tile inside the scores loop: the PV of iteration i, the
    # remaining q/k projections and the output projection all interleave
    # with iteration i+1's scores so ScalarE's exp stream never starves.
    pending = []

    def drain_one():
        if pending:
            pending.pop(0)()

    def scores_block(m, qc):
        kT_m, qT_m = kT_sb[m], qT_sb[m]
        qs = slice(qc * 512, (qc + 1) * 512)
        pts = []
        for kt in range(NKT):
            ks = slice(kt * P, (kt + 1) * P)
            sp = spsum.tile([P, 1024], F32, tag="sp", name="sp")
            # two heads row-packed: contraction rows 0-63 / 64-127
            nc.tensor.matmul(sp[:, 0:512],
                             lhsT=kT_m[0:CH, ks], rhs=qT_m[0:CH, qs])
            nc.tensor.matmul(sp[:, 512:1024],
                             lhsT=kT_m[CH:P, ks], rhs=qT_m[CH:P, qs])
            pt = ppool.tile([P, 1024], BF16, tag="pT", name="pT")
            nc.scalar.activation(pt[:], sp[:],
                                 mybir.ActivationFunctionType.Exp)
            pts.append(pt)
            drain_one()
        return pts

    def make_pv_groups(m, qc, pts):
        """8 pending groups of 4 accumulating PV matmuls; PSUM tiles are
        allocated lazily at the first group's emission."""
        aps = []

        def group(g):
            if not aps:
                aps.extend(apsum.tile([P, 512], F32, tag="ap", name="ap")
                           for _ in range(2))
            for kt in (2 * g, 2 * g + 1):
                for j in range(2):
                    h = 2 * m + j
                    nc.tensor.matmul(
                        aps[j][0:CH + 1, :],
                        lhsT=v_sb[kt][:, h, :],
                        rhs=pts[kt][:, j * 512:(j + 1) * 512],
                        start=(kt == 0), stop=(kt == NKT - 1))

        return [lambda g=g: group(g) for g in range(NKT // 2)], aps

    def norm_block(m, qc, aps):
        qs = slice(qc * 512, (qc + 1) * 512)
        for j in range(2):
            # partition_broadcast + custom DVE ops need base-0 APs on
            # HW: shift-copy the rowsum row 64 -> 0 first.
            rs = npool.tile([1, 512], F32, tag="rs", name="rs")
            nc.vector.tensor_copy(rs[0:1, :], aps[j][CH:CH + 1, :])
            rec = npool.tile([1, 512], F32, tag="rec", name="rec")
            nc.vector.reciprocal_approx_fast(rec[0:1, :], rs[0:1, :])
            bc = npool.tile([CH, 512], F32, tag="bc", name="bc")
            nc.gpsimd.partition_broadcast(bc[:], rec[0:1, :])
            # odd head writes lanes 64-127 (DVE partition shift)
            nc.vector.tensor_mul(attT_sb[m][j * CH:(j + 1) * CH, qs],
                                 aps[j][0:CH, :], bc[:])
            if dbg is not None and m == 0 and qc == 0 and j == 0:
                aps_sb = npool.tile([P, 512], F32, tag="dbg_aps",
                                    name="aps_sb")
                nc.vector.tensor_copy(aps_sb[:], aps[0][:])
                nc.sync.dma_start(dbg["d_aps"][:], aps_sb[:])
                nc.sync.dma_start(dbg["d_rec"][:], rec[0:1, :])
                nc.sync.dma_start(dbg["d_bc"][:], bc[:])

    # pending additions keyed by the iteration whose scores drain them
    adds = {
        (0, 2): [lambda t=i: emit_qk_proj("wq", 1, tcs=(t,)) for i in range(QC)],
        (0, 3): [lambda t=i: emit_qk_proj("wk", 1, tcs=(t,)) for i in range(QC)],
        (1, 0): [lambda t=i: emit_qk_proj("wq", 2, tcs=(t,)) for i in range(QC)]
              + [lambda t=i: emit_qk_proj("wk", 2, tcs=(t,)) for i in range(QC)],
        (1, 2): [lambda t=i: emit_qk_proj("wq", 3, tcs=(t,)) for i in range(QC)]
              + [lambda t=i: emit_qk_proj("wk", 3, tcs=(t,)) for i in range(QC)],
        (3, 1): [lambda t=tt: emit_out_proj(t) for tt in range(0, 4)],
        (3, 2): [lambda t=tt: emit_out_proj(t) for tt in range(4, 8)],
        (3, 3): [lambda t=tt: emit_out_proj(t) for tt in range(8, 12)],
    }

    # ---- prologue: minimal projection, then v under the first exps ---
    emit_qk_proj("wk", 0, tcs=(0,))
    emit_qk_proj("wq", 0, tcs=(0,))
    pending.extend([lambda t=i: emit_qk_proj("wk", 0, tcs=(t,))
                    for i in (1, 2, 3)])
    pending.append(lambda: emit_qk_proj("wq", 0, tcs=(1,)))
    pts_prev = scores_block(0, 0)
    if dbg is not None:
        nc.sync.dma_start(dbg["d_pT"][:], pts_prev[0][:])
    pending.extend([lambda t=i: emit_qk_proj("wq", 0, tcs=(t,))
                    for i in (2, 3)])
    for tt in range(TOKT):
        emit_v_proj(tt)

    # ---- software-pipelined main loop: scores(i+1) drains PV(i) + its
    # normalization (queued right behind the PV groups) -----------------
    iters = [(m, qc) for m in range(HEAD // 2) for qc in range(QC)]
    groups, aps = make_pv_groups(0, 0, pts_prev)
    pending.extend(groups)
    pending.append(lambda m=0, qc=0, a=aps: norm_block(m, qc, a))
    for (m, qc) in iters[1:]:
        pending.extend(adds.get((m, qc), []))
        pts = scores_block(m, qc)
        groups, aps = make_pv_groups(m, qc, pts)
        pending.extend(groups)
        pending.append(lambda m=m, qc=qc, a=aps: norm_block(m, qc, a))

    while pending:
        drain_one()

    for tt in range(12, 16):
        emit_out_proj(tt)

    if dbg is not None:
        nc.sync.dma_start(dbg["d_qT0"][:], qT_sb[0][:])
        nc.sync.dma_start(dbg["d_kT0"][:], kT_sb[0][:])
        nc.sync.dma_start(dbg["d_v0"][:], v_sb[0][:])
        nc.sync.dma_start(dbg["d_attT0"][:], attT_sb[0][0:CH, :])

    ctx.close()


def _numpy_fallback(x, Wq, bq, Wk, bk, Wv, bv, Wo, bo, t):
    """Exact replica of the reference in numpy (general path; only used for
    shapes/biases the tuned kernel doesn't handle)."""
    bt, n, c = x.shape
    b = bt // t
    head, ch = HEAD, c // HEAD
    hh = int(np.sqrt(n)); h2 = w2 = hh // 2
    q = x @ Wq.T + bq
    k = x @ Wk.T + bk
    v = x @ Wv.T + bv

    def win(z):
        z = z.reshape(b, t, 2, h2, 2, w2, head, ch)
        z = z.transpose(0, 2, 4, 6, 1, 3, 5, 7)
        return z.reshape(b, 4, head, t * h2 * w2, ch)

    q, k, v = win(q), win(k), win(v)
    s = (q @ k.transpose(0, 1, 2, 4, 3)) / np.sqrt(ch)
    s = s - s.max(-1, keepdims=True)
    p = np.exp(s)
    p = p / p.sum(-1, keepdims=True)
    att = p @ v
    att = att.reshape(b, 2, 2, head, t, h2, w2, ch)
    att = att.transpose(0, 4, 1, 5, 2, 6, 3, 7).reshape(bt, n, c)
    return (att @ Wo.T + bo).astype(x.dtype)


def kernel(x, Wq, bq, Wk, bk, Wv, bv, Wo, bo, t):
    global LAST_EXEC_NS, LAST_RESULTS, _COMPILED
    x = np.asarray(x); t = int(t)
    Wq, Wk, Wv, Wo = (np.asarray(a, np.float32) for a in (Wq, Wk, Wv, Wo))
    bq, bk, bv, bo = (np.asarray(a, np.float32) for a in (bq, bk, bv, bo))

    general = (
        x.shape != (16, 1024, 512) or t != 8
        or any(np.any(b) for b in (bq, bk, bv, bo))
    )
    if general:
        return _numpy_fallback(x, Wq, bq, Wk, bk, Wv, bv, Wo, bo, t)

    if _COMPILED is None:
        _COMPILED = _build_nc()
    nc = _COMPILED

    b = 16 // t
    # [b, 2, 2, t, 16, 16, c] spatial-window view of the token grid
    x6 = np.ascontiguousarray(
        x.reshape(b, t, 2, 16, 2, 16, C).transpose(0, 2, 4, 1, 3, 5, 6))
    wqT = (np.ascontiguousarray(Wq.T) * np.float32(1.0 / np.sqrt(CH))).astype(ml_dtypes.bfloat16)
    wkT = np.ascontiguousarray(Wk.T).astype(ml_dtypes.bfloat16)
    wvT = np.ascontiguousarray(Wv.T).astype(ml_dtypes.bfloat16)
    woT = np.ascontiguousarray(Wo.T).astype(ml_dtypes.bfloat16)

    in_maps = []
    for core in range(N_CORES):
        bb, wi, wj = core // 4, (core // 2) % 2, core % 2
        xw = x6[bb, wi, wj].reshape(S, C)
        in_maps.append({
            "xT": np.ascontiguousarray(xw.T).astype(ml_dtypes.bfloat16),
            "wqT": wqT, "wkT": wkT, "wvT": wvT, "woT": woT,
        })

    res = run_bass_kernel_spmd(nc, in_maps, list(range(N_CORES)))
    LAST_EXEC_NS = res.exec_time_ns
    LAST_RESULTS = res.results

    y6 = np.empty((b, 2, 2, t, 16, 16, C), np.float32)
    for core in range(N_CORES):
        bb, wi, wj = core // 4, (core // 2) % 2, core % 2
        y6[bb, wi, wj] = res.results[core]["out"].reshape(t, 16, 16, C)
    # invert the window view back to [bt, n, c]
    y = y6.transpose(0, 3, 1, 4, 2, 5, 6).reshape(16, 1024, C)
    return np.ascontiguousarray(y)
